# revision 5
# baseline (speedup 1.0000x reference)
"""TRN2 Bass kernel v3 for 16-head MHA (B=2, T=2048, D=1024).

Sharding: batch x head-quad across 8 cores (core = 4*b + hq handles batch b,
heads 4*hq..4*hq+3).  Per core: bf16 Q/K projections, bf16 V projection in
[token, dim] layout (no transposes; bv folded into the host-side output
bias), quadrant-packed bf16 S^T matmuls, softmax exp split across
ACT/DVE/GPSIMD (DVE+GP use an exp bit-trick producing bf16 bits), PV via
fp8 DoubleRow for head-pair 0 (PV8 flag per group) and bf16 for the rest,
normalization via a PE ones-x-recip broadcast matmul, bf16 output
projection, bf16 rank-256 partial written to DRAM.  Host sums 4 partials
per batch and adds bo + bv@Wo.
"""

import math
import numpy as np

import concourse.bass as bass
import concourse.mybir as mybir
import concourse.tile as tile
from concourse import bacc

FP32 = mybir.dt.float32
BF16 = mybir.dt.bfloat16
F8 = mybir.dt.float8e4
I8 = mybir.dt.int8
I16 = mybir.dt.int16
DR = mybir.MatmulPerfMode.DoubleRow
ACT_EXP = mybir.ActivationFunctionType.Exp
ACT_ID = mybir.ActivationFunctionType.Identity
OP = mybir.AluOpType

T = 2048          # tokens per core (one batch)
NG = T // 512     # q groups
LN2 = math.log(2)
# fp8 DoubleRow PV for head-pair 0, per group (flip entries to trade err/speed)
PV8 = [True, True, True, True]
# DVE/GP trick-exp constants (truncating float->int convert: +0.5 for rounding)
A16, B16 = 128.0 / LN2, 16256.0 + 0.5 - 5.5
# engine pattern for head-pair-1 exp units (hp0 fp8 units always go to ACT);
# 't' = bit-trick split DVE (PSUM->SBUF mul-add) + GPSIMD (SBUF clip->bf16 bits)
HP1_PAT = ['a', 't', 't', 't', 'a', 't', 't', 't', 'a', 't', 't', 't', 'a', 't', 't', 't']


def build(nc=None):
    if nc is None:
        nc = bacc.Bacc(
            "TRN2",
            target_bir_lowering=False,
            debug=False,
            enable_asserts=False,
            num_devices=8,
        )

    xqb = nc.dram_tensor("xqb", [128, 8, T], BF16, kind="ExternalInput")
    xkb = nc.dram_tensor("xkb", [128, 8, T], BF16, kind="ExternalInput")
    xvb = nc.dram_tensor("xvb", [128, 8, T], BF16, kind="ExternalInput")
    wqb = nc.dram_tensor("wqb", [128, 8, 256], BF16, kind="ExternalInput")
    wkb = nc.dram_tensor("wkb", [128, 8, 256], BF16, kind="ExternalInput")
    wvb = nc.dram_tensor("wvb", [128, 8, 256], BF16, kind="ExternalInput")
    wob = nc.dram_tensor("wob", [128, 2, 1024], BF16, kind="ExternalInput")
    bq8 = nc.dram_tensor("bq8", [128, 2], FP32, kind="ExternalInput")
    bkt = nc.dram_tensor("bkt", [128, 2], FP32, kind="ExternalInput")
    out = nc.dram_tensor("out", [T, 1024], BF16, kind="ExternalOutput")

    with tile.TileContext(nc) as tc:
        _emit(nc, tc, xqb, xkb, xvb, wqb, wkb, wvb, wob, bq8, bkt, out)

    nc.compile()
    return nc


class _E:
    pass


def _emit(nc, tc, xqb, xkb, xvb, wqb, wkb, wvb, wob, bq8, bkt, out):
    from contextlib import ExitStack

    E = _E()
    E.nc = nc
    E.ucount = 0   # hp1 exp unit counter
    E.opc = 0      # outproj chunks already emitted for pend[0]

    with ExitStack() as ctx:
        const = ctx.enter_context(tc.tile_pool(name="const", bufs=1))
        big = ctx.enter_context(tc.tile_pool(name="big", bufs=1))
        E.pt8_pool = ctx.enter_context(tc.tile_pool(name="pt8", bufs=3))
        E.ptb_pool = ctx.enter_context(tc.tile_pool(name="ptb", bufs=3))
        E.t_pool = ctx.enter_context(tc.tile_pool(name="tt", bufs=3))
        E.rc_pool = ctx.enter_context(tc.tile_pool(name="rc", bufs=2))
        E.ostg_pool = ctx.enter_context(tc.tile_pool(name="ostg", bufs=3))
        # PSUM: po 2 + st 2x2 + ctx 2x1 = 8 banks
        E.po_ps = ctx.enter_context(tc.tile_pool(name="po_ps", bufs=2, space="PSUM"))
        E.st_ps = ctx.enter_context(tc.tile_pool(name="st_ps", bufs=2, space="PSUM"))
        E.ctx_ps = ctx.enter_context(tc.tile_pool(name="ctx_ps", bufs=2, space="PSUM"))

        # ---- constants / weights ----
        wk_sb = const.tile([128, 8, 256], BF16, tag="wk")
        wq_sb = const.tile([128, 8, 256], BF16, tag="wq")
        wv_sb = const.tile([128, 8, 256], BF16, tag="wv")
        E.wo_sb = const.tile([128, 2, 1024], BF16, tag="wo")
        bq_sb = const.tile([128, 2], FP32, tag="bq")
        bk_sb = const.tile([128, 2], FP32, tag="bk")
        E.ebias = const.tile([128, 1], FP32, tag="ebias")
        E.ones = const.tile([128, 64], BF16, tag="ones")
        nc.sync.dma_start(wk_sb[:], wkb.ap())
        nc.sync.dma_start(wq_sb[:], wqb.ap())
        nc.sync.dma_start(wv_sb[:], wvb.ap())
        nc.sync.dma_start(E.wo_sb[:], wob.ap())
        nc.sync.dma_start(bq_sb[:], bq8.ap())
        nc.sync.dma_start(bk_sb[:], bkt.ap())
        nc.gpsimd.memset(E.ebias[:], float(-2 * LN2))
        nc.gpsimd.memset(E.ones[:], 1.0)

        # ---- persistent activations ----
        xk_sb = big.tile([128, 8, T], BF16, tag="xk")
        xq_sb = big.tile([128, 8, T], BF16, tag="xq")
        xv_sb = big.tile([128, 8, T], BF16, tag="xv")
        E.qT = big.tile([128, 2, T], BF16, tag="qT")
        E.kT = big.tile([128, 2, T], BF16, tag="kT")
        E.ctxT = big.tile([128, 2, T], BF16, tag="ctxT")
        # v8: [p, c2, hl, i, 65] fp8 (heads 0,1); vb: [p, tk, h, 65] bf16 (all 4)
        E.v8 = big.tile([128, 8, 2, 2, 80], F8, tag="v8")  # 80: 16B-aligned DoubleRow pair stride
        E.vb = big.tile([128, 16, 4, 65], BF16, tag="vb")

        # chunked input DMAs so projections start before full tensors land
        for kc in range(8):
            nc.sync.dma_start(xk_sb[:, kc, :], xkb.ap()[:, kc, :])
        for kc in range(8):
            nc.sync.dma_start(xq_sb[:, kc, :], xqb.ap()[:, kc, :])
        for kc in range(8):
            nc.sync.dma_start(xv_sb[:, kc, :], xvb.ap()[:, kc, :])
        nc.gpsimd.memset(E.v8[:, :, :, :, 64], 1.0)
        nc.gpsimd.memset(E.vb[:, :, :, 64], 1.0)

        # ---- Q/K projections (bf16) ----
        def qkproj(x_sb, w_sb, dstT, bias_sb, scale):
            for s in range(2):
                for t in range(4):
                    ps = E.po_ps.tile([128, 512], FP32, tag="po", name="qk_ps")
                    for kc in range(8):
                        nc.tensor.matmul(
                            ps[:],
                            w_sb[:, kc, s * 128:(s + 1) * 128],
                            x_sb[:, kc, t * 512:(t + 1) * 512],
                            start=(kc == 0), stop=(kc == 7),
                        )
                    nc.scalar.activation(
                        dstT[:, s, t * 512:(t + 1) * 512], ps[:], ACT_ID,
                        bias=bias_sb[:, s:s + 1], scale=scale)

        qkproj(xk_sb, wk_sb, E.kT, bk_sb, 1.0)
        qkproj(xq_sb, wq_sb, E.qT, bq_sb, 0.125)

        # ---- V projection (bf16, direct [tok, dim] layout) ----
        for tc_i in range(16):
            vp = E.po_ps.tile([128, 256], FP32, tag="po", name="v_ps")
            for kc in range(8):
                nc.tensor.matmul(
                    vp[:],
                    xv_sb[:, kc, tc_i * 128:(tc_i + 1) * 128],
                    wv_sb[:, kc, :],
                    start=(kc == 0), stop=(kc == 7),
                )
            c2, i = tc_i // 2, tc_i % 2
            nc.vector.tensor_copy(E.vb[:, tc_i, :, 0:64], vp[:])
            nc.vector.tensor_copy(E.v8[:, c2, :, i, 0:64], vp[:, 0:128])

        # ---- attention groups ----
        pend = []
        for g in range(NG):
            _group(E, g, pend, out)
        while pend:
            _outproj(E, pend.pop(0), out, range(8))


def _exp_unit(E, st, fp8, pt8, ptb, i):
    """Exp of one st tile [128, 2, 512] into pt slot i."""
    nc = E.nc
    if fp8:
        nc.scalar.activation(pt8[:, :, i, :], st[:], ACT_EXP,
                             bias=E.ebias[:], scale=1.0)
        return
    dst = ptb[:, :, i, :]
    eng = HP1_PAT[E.ucount % len(HP1_PAT)]
    E.ucount += 1
    if eng == 'a':
        nc.scalar.activation(dst, st[:], ACT_EXP, scale=1.0)
    else:
        t = E.t_pool.tile([128, 2, 512], FP32, tag="tt", name="t")
        nc.vector.tensor_scalar(t[:], st[:], float(A16), float(B16),
                                op0=OP.mult, op1=OP.add)
        nc.gpsimd.tensor_scalar(dst.bitcast(I16), t[:], 0.0, 32639.0,
                                op0=OP.max, op1=OP.min)


def _group(E, g, pend, out):
    nc = E.nc
    q0 = g * 512
    for hp in range(2):
        fp8 = (hp == 0) and PV8[g]
        ctx2 = [
            E.ctx_ps.tile([65, 512], FP32, tag="ctx", name=f"ctx{hl}")
            for hl in range(2)
        ]
        for c2 in range(8):
            pt8 = ptb = None
            if fp8:
                pt8 = E.pt8_pool.tile([128, 2, 2, 512], F8, tag="pt8", name="pt8")
            else:
                ptb = E.ptb_pool.tile([128, 2, 2, 512], BF16, tag="ptb", name="ptb")
            for i in range(2):
                tk = c2 * 2 + i
                st = E.st_ps.tile([128, 2, 512], FP32, tag="st", name="st")
                for hl in range(2):
                    nc.tensor.matmul(
                        st[:, hl, :],
                        E.kT[hl * 64:(hl + 1) * 64, hp, tk * 128:(tk + 1) * 128],
                        E.qT[hl * 64:(hl + 1) * 64, hp, q0:q0 + 512],
                        start=True, stop=True,
                    )
                _exp_unit(E, st, fp8, pt8, ptb, i)
                if not fp8:
                    for hl in range(2):
                        nc.tensor.matmul(
                            ctx2[hl][:],
                            E.vb[:, tk, hp * 2 + hl, :],
                            ptb[:, hl, i, :],
                            start=(tk == 0), stop=(tk == 15),
                        )
            if fp8:
                for hl in range(2):
                    nc.tensor.matmul(
                        ctx2[hl][:],
                        E.v8[:, c2, hl, :, 0:65],
                        pt8[:, hl, :, :],
                        start=(c2 == 0), stop=(c2 == 7), perf_mode=DR,
                    )
            # keep PE fed: one deferred outproj chunk every other c2
            if pend and (c2 % 4 == 1):
                _outproj(E, pend[0], out, [E.opc])
                E.opc += 1

        # ---- normalize + drain ctx for this head pair ----
        recipb = E.po_ps.tile([128, 512], FP32, tag="po", name="recipb")
        for hl in range(2):
            rsrc = E.rc_pool.tile([1, 512], FP32, tag="rs", name=f"rs{hl}")
            rcpf = E.rc_pool.tile([1, 512], FP32, tag="rff", name=f"rff{hl}")
            rcp = E.rc_pool.tile([1, 512], BF16, tag="rf", name=f"rf{hl}")
            nc.vector.tensor_copy(rsrc[0:1, :], ctx2[hl][64:65, :])
            nc.vector.reciprocal_approx_fast(rcpf[0:1, :], rsrc[0:1, :])
            nc.vector.tensor_copy(rcp[0:1, :], rcpf[0:1, :])
            nc.tensor.matmul(
                recipb[hl * 64:(hl + 1) * 64, :],
                E.ones[0:1, :],
                rcp[0:1, :],
                start=True, stop=True,
            )
        rb_sb = E.rc_pool.tile([128, 512], BF16, tag="rbsb", name="rb_sb")
        nc.vector.tensor_copy(rb_sb[:], recipb[:])
        for hl in range(2):
            nc.vector.tensor_tensor(
                E.ctxT[hl * 64:(hl + 1) * 64, hp, q0:q0 + 512],
                ctx2[hl][0:64, :],
                rb_sb[hl * 64:(hl + 1) * 64, :],
                op=OP.mult,
            )
    # finish any outproj chunks of the previous group not yet emitted
    if pend:
        gprev = pend.pop(0)
        _outproj(E, gprev, out, range(E.opc, 8))
    pend.append(g)
    E.opc = 0


def _outproj(E, g, out, chunks):
    nc = E.nc
    q0 = g * 512
    for ch in chunks:
        tc4, hh = ch // 2, ch % 2
        t0 = q0 + tc4 * 128
        ops = E.po_ps.tile([128, 512], FP32, tag="po", name="ops")
        for s in range(2):
            nc.tensor.matmul(
                ops[:],
                E.ctxT[:, s, t0:t0 + 128],
                E.wo_sb[:, s, hh * 512:(hh + 1) * 512],
                start=(s == 0), stop=(s == 1),
            )
        ostg = E.ostg_pool.tile([128, 512], BF16, tag="ostg")
        nc.vector.tensor_copy(ostg[:], ops[:])
        nc.sync.dma_start(out.ap()[t0:t0 + 128, hh * 512:(hh + 1) * 512], ostg[:])


# ---------------- host-side helpers ----------------

def core_inputs(q, k, v, Wq, bq, Wk, bk, Wv, bv, Wo, core):
    import ml_dtypes
    bf = ml_dtypes.bfloat16
    f8 = ml_dtypes.float8_e4m3  # noqa: F841 (fp8 staging handled on device)
    b, hq = core // 4, core % 4
    sl = slice(hq * 256, (hq + 1) * 256)

    def kc8(x):   # [1024, N] -> [128, 8, N]
        return np.ascontiguousarray(
            x.reshape(8, 128, x.shape[1]).transpose(1, 0, 2))

    return {
        "xqb": kc8(np.asarray(q[b], np.float32).T).astype(bf),
        "xkb": kc8(np.asarray(k[b], np.float32).T).astype(bf),
        "xvb": kc8(np.asarray(v[b], np.float32).T).astype(bf),
        "wqb": kc8(np.ascontiguousarray(Wq[:, sl])).astype(bf),
        "wkb": kc8(np.ascontiguousarray(Wk[:, sl])).astype(bf),
        "wvb": kc8(np.ascontiguousarray(Wv[:, sl])).astype(bf),
        "wob": np.ascontiguousarray(
            Wo[sl, :].reshape(2, 128, 1024).transpose(1, 0, 2)).astype(bf),
        "bq8": np.ascontiguousarray(
            (bq[sl] / 8.0).reshape(2, 128).T).astype(np.float32),
        "bkt": np.ascontiguousarray(bk[sl].reshape(2, 128).T).astype(np.float32),
    }


def shared_inputs(q, k, v):
    return {}


# ---------------- public entry point ----------------

_NC_CACHE = []


def _get_nc():
    if not _NC_CACHE:
        _NC_CACHE.append(build())
    return _NC_CACHE[0]


def kernel(q, k, v, Wq, bq, Wk, bk, Wv, bv, Wo, bo):
    from concourse import bass_utils

    args = [np.asarray(a, np.float32) for a in (q, k, v, Wq, bq, Wk, bk, Wv, bv, Wo)]
    q, k, v, Wq, bq, Wk, bk, Wv, bv, Wo = args
    bo = np.asarray(bo, np.float32)

    nc = _get_nc()
    in_maps = [core_inputs(q, k, v, Wq, bq, Wk, bk, Wv, bv, Wo, core)
               for core in range(8)]
    res = bass_utils.run_bass_kernel_spmd(nc, in_maps, core_ids=list(range(8)))

    host_bias = bo.astype(np.float64) + bv.astype(np.float64) @ Wo.astype(np.float64)
    outp = np.zeros((2, T, 1024), np.float64)
    for core in range(8):
        outp[core // 4] += res.results[core]["out"].astype(np.float64)
    return (outp + host_bias).astype(np.float32)


# revision 7
# speedup vs baseline: 3.6076x; 3.6076x over previous
"""TRN2 Bass kernel v3 for 16-head MHA (B=2, T=2048, D=1024).

Sharding: batch x head-quad across 8 cores (core = 4*b + hq handles batch b,
heads 4*hq..4*hq+3).  Per core: bf16 Q/K projections, bf16 V projection in
[token, dim] layout (no transposes; bv folded into the host-side output
bias), quadrant-packed bf16 S^T matmuls, softmax exp split across
ACT/DVE/GPSIMD (DVE+GP use an exp bit-trick producing bf16 bits), PV via
fp8 DoubleRow for head-pair 0 (PV8 flag per group) and bf16 for the rest,
normalization via a PE ones-x-recip broadcast matmul, bf16 output
projection, bf16 rank-256 partial written to DRAM.  Host sums 4 partials
per batch and adds bo + bv@Wo.
"""

import math
import numpy as np

import concourse.bass as bass
import concourse.mybir as mybir
import concourse.tile as tile
from concourse import bacc

FP32 = mybir.dt.float32
BF16 = mybir.dt.bfloat16
F8 = mybir.dt.float8e4
I8 = mybir.dt.int8
I16 = mybir.dt.int16
DR = mybir.MatmulPerfMode.DoubleRow
ACT_EXP = mybir.ActivationFunctionType.Exp
ACT_ID = mybir.ActivationFunctionType.Identity
OP = mybir.AluOpType

T = 2048          # tokens per core (one batch)
NG = T // 512     # q groups
LN2 = math.log(2)
# fp8 DoubleRow PV for head-pair 0, per group (flip entries to trade err/speed)
PV8 = [True, True, False, False]
# DVE trick-exp constants (truncating float->int convert: +0.5 for rounding);
# S in [-7, 7] keeps t inside [0, 32639] so no clip is needed
A16, B16 = 128.0 / LN2, 16256.0 + 0.5 - 5.5
# engine pattern for head-pair-1 exp units (hp0 fp8 units always go to ACT);
# 't' = single-op DVE bit-trick (mul-add, int16 convert = bf16 bits)
HP1_PAT = ['a', 't'] * 8


def build(nc=None):
    if nc is None:
        nc = bacc.Bacc(
            "TRN2",
            target_bir_lowering=False,
            debug=False,
            enable_asserts=False,
            num_devices=8,
        )

    xqb = nc.dram_tensor("xqb", [128, 8, T], BF16, kind="ExternalInput")
    xkb = nc.dram_tensor("xkb", [128, 8, T], BF16, kind="ExternalInput")
    xvb = nc.dram_tensor("xvb", [128, 8, T], BF16, kind="ExternalInput")
    wqb = nc.dram_tensor("wqb", [128, 8, 256], BF16, kind="ExternalInput")
    wkb = nc.dram_tensor("wkb", [128, 8, 256], BF16, kind="ExternalInput")
    wvb = nc.dram_tensor("wvb", [128, 8, 256], BF16, kind="ExternalInput")
    wob = nc.dram_tensor("wob", [128, 2, 1024], BF16, kind="ExternalInput")
    bq8 = nc.dram_tensor("bq8", [128, 2], FP32, kind="ExternalInput")
    bkt = nc.dram_tensor("bkt", [128, 2], FP32, kind="ExternalInput")
    out = nc.dram_tensor("out", [T, 1024], BF16, kind="ExternalOutput")

    with tile.TileContext(nc) as tc:
        _emit(nc, tc, xqb, xkb, xvb, wqb, wkb, wvb, wob, bq8, bkt, out)

    nc.compile()
    return nc


class _E:
    pass


def _emit(nc, tc, xqb, xkb, xvb, wqb, wkb, wvb, wob, bq8, bkt, out):
    from contextlib import ExitStack

    E = _E()
    E.nc = nc
    E.ucount = 0   # hp1 exp unit counter
    E.opc = 0      # outproj chunks already emitted for pend[0]

    with ExitStack() as ctx:
        const = ctx.enter_context(tc.tile_pool(name="const", bufs=1))
        big = ctx.enter_context(tc.tile_pool(name="big", bufs=1))
        E.pt8_pool = ctx.enter_context(tc.tile_pool(name="pt8", bufs=3))
        E.ptb_pool = ctx.enter_context(tc.tile_pool(name="ptb", bufs=3))
        E.rc_pool = ctx.enter_context(tc.tile_pool(name="rc", bufs=2))
        E.ostg_pool = ctx.enter_context(tc.tile_pool(name="ostg", bufs=3))
        # PSUM: po 2 + st 2x2 + ctx 2x1 = 8 banks
        E.po_ps = ctx.enter_context(tc.tile_pool(name="po_ps", bufs=2, space="PSUM"))
        E.st_ps = ctx.enter_context(tc.tile_pool(name="st_ps", bufs=2, space="PSUM"))
        E.ctx_ps = ctx.enter_context(tc.tile_pool(name="ctx_ps", bufs=2, space="PSUM"))

        # ---- constants / weights ----
        wk_sb = const.tile([128, 8, 256], BF16, tag="wk")
        wq_sb = const.tile([128, 8, 256], BF16, tag="wq")
        wv_sb = const.tile([128, 8, 256], BF16, tag="wv")
        E.wo_sb = const.tile([128, 2, 1024], BF16, tag="wo")
        bq_sb = const.tile([128, 2], FP32, tag="bq")
        bk_sb = const.tile([128, 2], FP32, tag="bk")
        E.ebias = const.tile([128, 1], FP32, tag="ebias")
        E.ones = const.tile([128, 64], BF16, tag="ones")
        nc.sync.dma_start(wk_sb[:], wkb.ap())
        nc.sync.dma_start(wq_sb[:], wqb.ap())
        nc.sync.dma_start(wv_sb[:], wvb.ap())
        nc.sync.dma_start(E.wo_sb[:], wob.ap())
        nc.sync.dma_start(bq_sb[:], bq8.ap())
        nc.sync.dma_start(bk_sb[:], bkt.ap())
        nc.gpsimd.memset(E.ebias[:], float(-2 * LN2))
        nc.gpsimd.memset(E.ones[:], 1.0)

        # ---- persistent activations ----
        xk_sb = big.tile([128, 8, T], BF16, tag="xk")
        xq_sb = big.tile([128, 8, T], BF16, tag="xq")
        xv_sb = big.tile([128, 8, T], BF16, tag="xv")
        E.qT = big.tile([128, 2, T], BF16, tag="qT")
        E.kT = big.tile([128, 2, T], BF16, tag="kT")
        E.ctxT = big.tile([128, 2, T], BF16, tag="ctxT")
        # v8: [p, c2, hl, i, 65] fp8 (heads 0,1); vb: [p, tk, h, 65] bf16 (all 4)
        E.v8 = big.tile([128, 8, 2, 2, 80], F8, tag="v8")  # 80: 16B-aligned DoubleRow pair stride
        E.vb = big.tile([128, 16, 4, 65], BF16, tag="vb")

        # chunked input DMAs so projections start before full tensors land
        for kc in range(8):
            nc.sync.dma_start(xk_sb[:, kc, :], xkb.ap()[:, kc, :])
        for kc in range(8):
            nc.sync.dma_start(xq_sb[:, kc, :], xqb.ap()[:, kc, :])
        for kc in range(8):
            nc.sync.dma_start(xv_sb[:, kc, :], xvb.ap()[:, kc, :])
        nc.gpsimd.memset(E.v8[:, :, :, :, 64], 1.0)
        nc.gpsimd.memset(E.vb[:, :, :, 64], 1.0)

        # ---- Q/K projections (bf16) ----
        def qkproj(x_sb, w_sb, dstT, bias_sb, scale):
            for s in range(2):
                for t in range(4):
                    ps = E.po_ps.tile([128, 512], FP32, tag="po", name="qk_ps")
                    for kc in range(8):
                        nc.tensor.matmul(
                            ps[:],
                            w_sb[:, kc, s * 128:(s + 1) * 128],
                            x_sb[:, kc, t * 512:(t + 1) * 512],
                            start=(kc == 0), stop=(kc == 7),
                        )
                    nc.scalar.activation(
                        dstT[:, s, t * 512:(t + 1) * 512], ps[:], ACT_ID,
                        bias=bias_sb[:, s:s + 1], scale=scale)

        qkproj(xk_sb, wk_sb, E.kT, bk_sb, 1.0)
        qkproj(xq_sb, wq_sb, E.qT, bq_sb, 0.125)

        # ---- V projection (bf16, direct [tok, dim] layout) ----
        for tc_i in range(16):
            vp = E.po_ps.tile([128, 256], FP32, tag="po", name="v_ps")
            for kc in range(8):
                nc.tensor.matmul(
                    vp[:],
                    xv_sb[:, kc, tc_i * 128:(tc_i + 1) * 128],
                    wv_sb[:, kc, :],
                    start=(kc == 0), stop=(kc == 7),
                )
            c2, i = tc_i // 2, tc_i % 2
            nc.vector.tensor_copy(E.vb[:, tc_i, :, 0:64], vp[:])
            nc.vector.tensor_copy(E.v8[:, c2, :, i, 0:64], vp[:, 0:128])

        # ---- attention groups ----
        pend = []
        for g in range(NG):
            _group(E, g, pend, out)
        while pend:
            _outproj(E, pend.pop(0), out, range(8))


def _exp_unit(E, st, fp8, pt8, ptb, i):
    """Exp of one st tile [128, 2, 512] into pt slot i."""
    nc = E.nc
    if fp8:
        nc.scalar.activation(pt8[:, :, i, :], st[:], ACT_EXP,
                             bias=E.ebias[:], scale=1.0)
        return
    dst = ptb[:, :, i, :]
    eng = HP1_PAT[E.ucount % len(HP1_PAT)]
    E.ucount += 1
    if eng == 'a':
        nc.scalar.activation(dst, st[:], ACT_EXP, scale=1.0)
    else:
        nc.vector.tensor_scalar(dst.bitcast(I16), st[:], float(A16), float(B16),
                                op0=OP.mult, op1=OP.add)


def _group(E, g, pend, out):
    nc = E.nc
    q0 = g * 512
    for hp in range(2):
        fp8 = (hp == 0) and PV8[g]
        ctx2 = [
            E.ctx_ps.tile([65, 512], FP32, tag="ctx", name=f"ctx{hl}")
            for hl in range(2)
        ]
        def pv(c2, pt):
            """PV matmuls for double-chunk c2 (software-pipelined one c2 behind)."""
            if fp8:
                for hl in range(2):
                    nc.tensor.matmul(
                        ctx2[hl][:],
                        E.v8[:, c2, hl, :, 0:65],
                        pt[:, hl, :, :],
                        start=(c2 == 0), stop=(c2 == 7), perf_mode=DR,
                    )
            else:
                for i in range(2):
                    tk = c2 * 2 + i
                    for hl in range(2):
                        nc.tensor.matmul(
                            ctx2[hl][:],
                            E.vb[:, tk, hp * 2 + hl, :],
                            pt[:, hl, i, :],
                            start=(tk == 0), stop=(tk == 15),
                        )

        prev = None
        for c2 in range(8):
            if fp8:
                pt = E.pt8_pool.tile([128, 2, 2, 512], F8, tag="pt8", name="pt8")
            else:
                pt = E.ptb_pool.tile([128, 2, 2, 512], BF16, tag="ptb", name="ptb")
            for i in range(2):
                tk = c2 * 2 + i
                st = E.st_ps.tile([128, 2, 512], FP32, tag="st", name="st")
                for hl in range(2):
                    nc.tensor.matmul(
                        st[:, hl, :],
                        E.kT[hl * 64:(hl + 1) * 64, hp, tk * 128:(tk + 1) * 128],
                        E.qT[hl * 64:(hl + 1) * 64, hp, q0:q0 + 512],
                        start=True, stop=True,
                    )
                _exp_unit(E, st, fp8, pt if fp8 else None, None if fp8 else pt, i)
            if prev is not None:
                pv(*prev)
            prev = (c2, pt)
            # keep PE fed: one deferred outproj chunk every other c2
            if pend and (c2 % 4 == 1):
                _outproj(E, pend[0], out, [E.opc])
                E.opc += 1
        pv(*prev)

        # ---- normalize + drain ctx for this head pair ----
        recipb = E.po_ps.tile([128, 512], FP32, tag="po", name="recipb")
        for hl in range(2):
            rsrc = E.rc_pool.tile([1, 512], FP32, tag="rs", name=f"rs{hl}")
            rcpf = E.rc_pool.tile([1, 512], FP32, tag="rff", name=f"rff{hl}")
            rcp = E.rc_pool.tile([1, 512], BF16, tag="rf", name=f"rf{hl}")
            nc.vector.tensor_copy(rsrc[0:1, :], ctx2[hl][64:65, :])
            nc.vector.reciprocal_approx_fast(rcpf[0:1, :], rsrc[0:1, :])
            nc.vector.tensor_copy(rcp[0:1, :], rcpf[0:1, :])
            nc.tensor.matmul(
                recipb[hl * 64:(hl + 1) * 64, :],
                E.ones[0:1, :],
                rcp[0:1, :],
                start=True, stop=True,
            )
        rb_sb = E.rc_pool.tile([128, 512], BF16, tag="rbsb", name="rb_sb")
        nc.vector.tensor_copy(rb_sb[:], recipb[:])
        for hl in range(2):
            nc.vector.tensor_tensor(
                E.ctxT[hl * 64:(hl + 1) * 64, hp, q0:q0 + 512],
                ctx2[hl][0:64, :],
                rb_sb[hl * 64:(hl + 1) * 64, :],
                op=OP.mult,
            )
    # finish any outproj chunks of the previous group not yet emitted
    if pend:
        gprev = pend.pop(0)
        _outproj(E, gprev, out, range(E.opc, 8))
    pend.append(g)
    E.opc = 0


def _outproj(E, g, out, chunks):
    nc = E.nc
    q0 = g * 512
    for ch in chunks:
        tc4, hh = ch // 2, ch % 2
        t0 = q0 + tc4 * 128
        ops = E.po_ps.tile([128, 512], FP32, tag="po", name="ops")
        for s in range(2):
            nc.tensor.matmul(
                ops[:],
                E.ctxT[:, s, t0:t0 + 128],
                E.wo_sb[:, s, hh * 512:(hh + 1) * 512],
                start=(s == 0), stop=(s == 1),
            )
        ostg = E.ostg_pool.tile([128, 512], BF16, tag="ostg")
        nc.vector.tensor_copy(ostg[:], ops[:])
        nc.sync.dma_start(out.ap()[t0:t0 + 128, hh * 512:(hh + 1) * 512], ostg[:])


# ---------------- host-side helpers ----------------

def core_inputs(q, k, v, Wq, bq, Wk, bk, Wv, bv, Wo, core):
    import ml_dtypes
    bf = ml_dtypes.bfloat16
    f8 = ml_dtypes.float8_e4m3  # noqa: F841 (fp8 staging handled on device)
    b, hq = core // 4, core % 4
    sl = slice(hq * 256, (hq + 1) * 256)

    def kc8(x):   # [1024, N] -> [128, 8, N]
        return np.ascontiguousarray(
            x.reshape(8, 128, x.shape[1]).transpose(1, 0, 2))

    return {
        "xqb": kc8(np.asarray(q[b], np.float32).T).astype(bf),
        "xkb": kc8(np.asarray(k[b], np.float32).T).astype(bf),
        "xvb": kc8(np.asarray(v[b], np.float32).T).astype(bf),
        "wqb": kc8(np.ascontiguousarray(Wq[:, sl])).astype(bf),
        "wkb": kc8(np.ascontiguousarray(Wk[:, sl])).astype(bf),
        "wvb": kc8(np.ascontiguousarray(Wv[:, sl])).astype(bf),
        "wob": np.ascontiguousarray(
            Wo[sl, :].reshape(2, 128, 1024).transpose(1, 0, 2)).astype(bf),
        "bq8": np.ascontiguousarray(
            (bq[sl] / 8.0).reshape(2, 128).T).astype(np.float32),
        "bkt": np.ascontiguousarray(bk[sl].reshape(2, 128).T).astype(np.float32),
    }


def shared_inputs(q, k, v):
    return {}


# ---------------- public entry point ----------------

_NC_CACHE = []


def _get_nc():
    if not _NC_CACHE:
        _NC_CACHE.append(build())
    return _NC_CACHE[0]


def kernel(q, k, v, Wq, bq, Wk, bk, Wv, bv, Wo, bo):
    from concourse import bass_utils

    args = [np.asarray(a, np.float32) for a in (q, k, v, Wq, bq, Wk, bk, Wv, bv, Wo)]
    q, k, v, Wq, bq, Wk, bk, Wv, bv, Wo = args
    bo = np.asarray(bo, np.float32)

    nc = _get_nc()
    in_maps = [core_inputs(q, k, v, Wq, bq, Wk, bk, Wv, bv, Wo, core)
               for core in range(8)]
    res = bass_utils.run_bass_kernel_spmd(nc, in_maps, core_ids=list(range(8)))

    host_bias = bo.astype(np.float64) + bv.astype(np.float64) @ Wo.astype(np.float64)
    outp = np.zeros((2, T, 1024), np.float64)
    for core in range(8):
        outp[core // 4] += res.results[core]["out"].astype(np.float64)
    return (outp + host_bias).astype(np.float32)


# revision 8
# speedup vs baseline: 3.7230x; 1.0320x over previous
"""TRN2 Bass kernel v3 for 16-head MHA (B=2, T=2048, D=1024).

Sharding: batch x head-quad across 8 cores (core = 4*b + hq handles batch b,
heads 4*hq..4*hq+3).  Per core: bf16 Q/K projections, bf16 V projection in
[token, dim] layout (no transposes; bv folded into the host-side output
bias), quadrant-packed bf16 S^T matmuls, softmax exp split across
ACT/DVE/GPSIMD (DVE+GP use an exp bit-trick producing bf16 bits), PV via
fp8 DoubleRow for head-pair 0 (PV8 flag per group) and bf16 for the rest,
normalization via a PE ones-x-recip broadcast matmul, bf16 output
projection, bf16 rank-256 partial written to DRAM.  Host sums 4 partials
per batch and adds bo + bv@Wo.
"""

import math
import numpy as np

import concourse.bass as bass
import concourse.mybir as mybir
import concourse.tile as tile
from concourse import bacc

FP32 = mybir.dt.float32
BF16 = mybir.dt.bfloat16
F8 = mybir.dt.float8e4
I8 = mybir.dt.int8
I16 = mybir.dt.int16
DR = mybir.MatmulPerfMode.DoubleRow
ACT_EXP = mybir.ActivationFunctionType.Exp
ACT_ID = mybir.ActivationFunctionType.Identity
OP = mybir.AluOpType

T = 2048          # tokens per core (one batch)
NG = T // 512     # q groups
LN2 = math.log(2)
# fp8 DoubleRow PV for head-pair 0, per group (flip entries to trade err/speed)
PV8 = [True, True, False, False]
# DVE trick-exp constants (truncating float->int convert: +0.5 for rounding);
# S in [-7, 7] keeps t inside [0, 32639] so no clip is needed
A16, B16 = 128.0 / LN2, 16256.0 + 0.5 - 5.5
# engine pattern for head-pair-1 exp units (hp0 fp8 units always go to ACT);
# 't' = single-op DVE bit-trick (mul-add, int16 convert = bf16 bits)
HP1_PAT = ['a', 't'] * 8


def build(nc=None):
    if nc is None:
        nc = bacc.Bacc(
            "TRN2",
            target_bir_lowering=False,
            debug=False,
            enable_asserts=False,
            num_devices=8,
        )

    xqb = nc.dram_tensor("xqb", [128, 8, T], BF16, kind="ExternalInput")
    xkb = nc.dram_tensor("xkb", [128, 8, T], BF16, kind="ExternalInput")
    xvb = nc.dram_tensor("xvb", [128, 8, T], BF16, kind="ExternalInput")
    wqb = nc.dram_tensor("wqb", [128, 8, 256], BF16, kind="ExternalInput")
    wkb = nc.dram_tensor("wkb", [128, 8, 256], BF16, kind="ExternalInput")
    wvb = nc.dram_tensor("wvb", [128, 8, 256], BF16, kind="ExternalInput")
    wob = nc.dram_tensor("wob", [128, 2, 1024], BF16, kind="ExternalInput")
    bq8 = nc.dram_tensor("bq8", [128, 2], FP32, kind="ExternalInput")
    bkt = nc.dram_tensor("bkt", [128, 2], FP32, kind="ExternalInput")
    out = nc.dram_tensor("out", [T, 1024], BF16, kind="ExternalOutput")

    with tile.TileContext(nc) as tc:
        _emit(nc, tc, xqb, xkb, xvb, wqb, wkb, wvb, wob, bq8, bkt, out)

    nc.compile()
    return nc


class _E:
    pass


def _emit(nc, tc, xqb, xkb, xvb, wqb, wkb, wvb, wob, bq8, bkt, out):
    from contextlib import ExitStack

    E = _E()
    E.nc = nc
    E.ucount = 0   # hp1 exp unit counter
    E.opc = 0      # outproj chunks already emitted for pend[0]

    with ExitStack() as ctx:
        const = ctx.enter_context(tc.tile_pool(name="const", bufs=1))
        big = ctx.enter_context(tc.tile_pool(name="big", bufs=1))
        E.pt8_pool = ctx.enter_context(tc.tile_pool(name="pt8", bufs=3))
        E.ptb_pool = ctx.enter_context(tc.tile_pool(name="ptb", bufs=4))
        E.rc_pool = ctx.enter_context(tc.tile_pool(name="rc", bufs=2))
        E.ostg_pool = ctx.enter_context(tc.tile_pool(name="ostg", bufs=3))
        # PSUM: po 2 + st 2x2 + ctx 2x1 = 8 banks
        E.po_ps = ctx.enter_context(tc.tile_pool(name="po_ps", bufs=2, space="PSUM"))
        E.st_ps = ctx.enter_context(tc.tile_pool(name="st_ps", bufs=2, space="PSUM"))
        E.ctx_ps = ctx.enter_context(tc.tile_pool(name="ctx_ps", bufs=2, space="PSUM"))

        # ---- constants / weights ----
        wk_sb = const.tile([128, 8, 256], BF16, tag="wk")
        wq_sb = const.tile([128, 8, 256], BF16, tag="wq")
        wv_sb = const.tile([128, 8, 256], BF16, tag="wv")
        E.wo_sb = const.tile([128, 2, 1024], BF16, tag="wo")
        bq_sb = const.tile([128, 2], FP32, tag="bq")
        bk_sb = const.tile([128, 2], FP32, tag="bk")
        E.ebias = const.tile([128, 1], FP32, tag="ebias")
        E.ones = const.tile([128, 64], BF16, tag="ones")
        nc.sync.dma_start(wk_sb[:], wkb.ap())
        nc.gpsimd.memset(E.ebias[:], float(-2 * LN2))
        nc.gpsimd.memset(E.ones[:], 1.0)

        # ---- persistent activations ----
        xk_sb = big.tile([128, 8, T], BF16, tag="xk")
        xq_sb = big.tile([128, 8, T], BF16, tag="xq")
        xv_sb = big.tile([128, 8, T], BF16, tag="xv")
        E.qT = big.tile([128, 2, T], BF16, tag="qT")
        E.kT = big.tile([128, 2, T], BF16, tag="kT")
        E.ctxT = big.tile([128, 2, T], BF16, tag="ctxT")
        # v8: [p, c2, hl, i, 65] fp8 (heads 0,1); vb: [p, tk, h, 65] bf16 (all 4)
        E.v8 = big.tile([128, 8, 2, 2, 80], F8, tag="v8")  # 80: 16B-aligned DoubleRow pair stride
        E.vb = big.tile([128, 16, 4, 65], BF16, tag="vb")

        # chunked input DMAs so projections start before full tensors land
        for kc in range(8):
            nc.sync.dma_start(xk_sb[:, kc, :], xkb.ap()[:, kc, :])
        nc.sync.dma_start(bk_sb[:], bkt.ap())
        nc.sync.dma_start(wq_sb[:], wqb.ap())
        for kc in range(8):
            nc.sync.dma_start(xq_sb[:, kc, :], xqb.ap()[:, kc, :])
        nc.sync.dma_start(bq_sb[:], bq8.ap())
        nc.sync.dma_start(wv_sb[:], wvb.ap())
        for kc in range(8):
            nc.sync.dma_start(xv_sb[:, kc, :], xvb.ap()[:, kc, :])
        nc.sync.dma_start(E.wo_sb[:], wob.ap())
        nc.gpsimd.memset(E.v8[:, :, :, :, 64], 1.0)
        nc.gpsimd.memset(E.vb[:, :, :, 64], 1.0)

        # ---- Q/K projections (bf16) ----
        def qkproj(x_sb, w_sb, dstT, bias_sb, scale):
            for s in range(2):
                for t in range(4):
                    ps = E.po_ps.tile([128, 512], FP32, tag="po", name="qk_ps")
                    for kc in range(8):
                        nc.tensor.matmul(
                            ps[:],
                            w_sb[:, kc, s * 128:(s + 1) * 128],
                            x_sb[:, kc, t * 512:(t + 1) * 512],
                            start=(kc == 0), stop=(kc == 7),
                        )
                    nc.scalar.activation(
                        dstT[:, s, t * 512:(t + 1) * 512], ps[:], ACT_ID,
                        bias=bias_sb[:, s:s + 1], scale=scale)

        qkproj(xk_sb, wk_sb, E.kT, bk_sb, 1.0)
        qkproj(xq_sb, wq_sb, E.qT, bq_sb, 0.125)

        # ---- V projection (bf16, direct [tok, dim] layout) ----
        for tc_i in range(16):
            vp = E.po_ps.tile([128, 256], FP32, tag="po", name="v_ps")
            for kc in range(8):
                nc.tensor.matmul(
                    vp[:],
                    xv_sb[:, kc, tc_i * 128:(tc_i + 1) * 128],
                    wv_sb[:, kc, :],
                    start=(kc == 0), stop=(kc == 7),
                )
            c2, i = tc_i // 2, tc_i % 2
            nc.vector.tensor_copy(E.vb[:, tc_i, :, 0:64], vp[:])
            nc.vector.tensor_copy(E.v8[:, c2, :, i, 0:64], vp[:, 0:128])

        # ---- attention groups ----
        pend = []
        for g in range(NG):
            _group(E, g, pend, out)
        while pend:
            _outproj(E, pend.pop(0), out, range(8))


def _exp_unit(E, st, fp8, pt8, ptb, i):
    """Exp of one st tile [128, 2, 512] into pt slot i."""
    nc = E.nc
    if fp8:
        nc.scalar.activation(pt8[:, :, i, :], st[:], ACT_EXP,
                             bias=E.ebias[:], scale=1.0)
        return
    dst = ptb[:, :, i, :]
    eng = HP1_PAT[E.ucount % len(HP1_PAT)]
    E.ucount += 1
    if eng == 'a':
        nc.scalar.activation(dst, st[:], ACT_EXP, scale=1.0)
    else:
        nc.vector.tensor_scalar(dst.bitcast(I16), st[:], float(A16), float(B16),
                                op0=OP.mult, op1=OP.add)


def _group(E, g, pend, out):
    nc = E.nc
    q0 = g * 512
    for hp in range(2):
        fp8 = (hp == 0) and PV8[g]
        ctx2 = [
            E.ctx_ps.tile([65, 512], FP32, tag="ctx", name=f"ctx{hl}")
            for hl in range(2)
        ]
        def pv(c2, pt):
            """PV matmuls for double-chunk c2 (software-pipelined one c2 behind)."""
            if fp8:
                for hl in range(2):
                    nc.tensor.matmul(
                        ctx2[hl][:],
                        E.v8[:, c2, hl, :, 0:65],
                        pt[:, hl, :, :],
                        start=(c2 == 0), stop=(c2 == 7), perf_mode=DR,
                    )
            else:
                for i in range(2):
                    tk = c2 * 2 + i
                    for hl in range(2):
                        nc.tensor.matmul(
                            ctx2[hl][:],
                            E.vb[:, tk, hp * 2 + hl, :],
                            pt[:, hl, i, :],
                            start=(tk == 0), stop=(tk == 15),
                        )

        prev = None
        for c2 in range(8):
            if fp8:
                pt = E.pt8_pool.tile([128, 2, 2, 512], F8, tag="pt8", name="pt8")
            else:
                pt = E.ptb_pool.tile([128, 2, 2, 512], BF16, tag="ptb", name="ptb")
            for i in range(2):
                tk = c2 * 2 + i
                st = E.st_ps.tile([128, 2, 512], FP32, tag="st", name="st")
                for hl in range(2):
                    nc.tensor.matmul(
                        st[:, hl, :],
                        E.kT[hl * 64:(hl + 1) * 64, hp, tk * 128:(tk + 1) * 128],
                        E.qT[hl * 64:(hl + 1) * 64, hp, q0:q0 + 512],
                        start=True, stop=True,
                    )
                _exp_unit(E, st, fp8, pt if fp8 else None, None if fp8 else pt, i)
            if prev is not None:
                pv(*prev)
            prev = (c2, pt)
            # keep PE fed: one deferred outproj chunk every other c2
            if pend and (c2 % 4 == 1):
                _outproj(E, pend[0], out, [E.opc])
                E.opc += 1
        pv(*prev)

        # ---- normalize + drain ctx for this head pair ----
        recipb = E.po_ps.tile([128, 512], FP32, tag="po", name="recipb")
        rcps = []
        for hl in range(2):
            rsrc = E.rc_pool.tile([1, 512], FP32, tag="rs", name=f"rs{hl}")
            rcpf = E.rc_pool.tile([1, 512], FP32, tag="rff", name=f"rff{hl}")
            rcp = E.rc_pool.tile([1, 512], BF16, tag="rf", name=f"rf{hl}")
            nc.vector.tensor_copy(rsrc[0:1, :], ctx2[hl][64:65, :])
            nc.vector.reciprocal_approx_fast(rcpf[0:1, :], rsrc[0:1, :])
            nc.vector.tensor_copy(rcp[0:1, :], rcpf[0:1, :])
            rcps.append(rcp)
        for hl in range(2):
            nc.tensor.matmul(
                recipb[hl * 64:(hl + 1) * 64, :],
                E.ones[0:1, :],
                rcps[hl][0:1, :],
                start=True, stop=True,
            )
        rb_sb = E.rc_pool.tile([128, 512], BF16, tag="rbsb", name="rb_sb")
        nc.vector.tensor_copy(rb_sb[:], recipb[:])
        for hl in range(2):
            nc.vector.tensor_tensor(
                E.ctxT[hl * 64:(hl + 1) * 64, hp, q0:q0 + 512],
                ctx2[hl][0:64, :],
                rb_sb[hl * 64:(hl + 1) * 64, :],
                op=OP.mult,
            )
    # finish any outproj chunks of the previous group not yet emitted
    if pend:
        gprev = pend.pop(0)
        _outproj(E, gprev, out, range(E.opc, 8))
    pend.append(g)
    E.opc = 0


def _outproj(E, g, out, chunks):
    nc = E.nc
    q0 = g * 512
    for ch in chunks:
        tc4, hh = ch // 2, ch % 2
        t0 = q0 + tc4 * 128
        ops = E.po_ps.tile([128, 512], FP32, tag="po", name="ops")
        for s in range(2):
            nc.tensor.matmul(
                ops[:],
                E.ctxT[:, s, t0:t0 + 128],
                E.wo_sb[:, s, hh * 512:(hh + 1) * 512],
                start=(s == 0), stop=(s == 1),
            )
        ostg = E.ostg_pool.tile([128, 512], BF16, tag="ostg")
        nc.vector.tensor_copy(ostg[:], ops[:])
        nc.sync.dma_start(out.ap()[t0:t0 + 128, hh * 512:(hh + 1) * 512], ostg[:])


# ---------------- host-side helpers ----------------

def core_inputs(q, k, v, Wq, bq, Wk, bk, Wv, bv, Wo, core):
    import ml_dtypes
    bf = ml_dtypes.bfloat16
    f8 = ml_dtypes.float8_e4m3  # noqa: F841 (fp8 staging handled on device)
    b, hq = core // 4, core % 4
    sl = slice(hq * 256, (hq + 1) * 256)

    def kc8(x):   # [1024, N] -> [128, 8, N]
        return np.ascontiguousarray(
            x.reshape(8, 128, x.shape[1]).transpose(1, 0, 2))

    return {
        "xqb": kc8(np.asarray(q[b], np.float32).T).astype(bf),
        "xkb": kc8(np.asarray(k[b], np.float32).T).astype(bf),
        "xvb": kc8(np.asarray(v[b], np.float32).T).astype(bf),
        "wqb": kc8(np.ascontiguousarray(Wq[:, sl])).astype(bf),
        "wkb": kc8(np.ascontiguousarray(Wk[:, sl])).astype(bf),
        "wvb": kc8(np.ascontiguousarray(Wv[:, sl])).astype(bf),
        "wob": np.ascontiguousarray(
            Wo[sl, :].reshape(2, 128, 1024).transpose(1, 0, 2)).astype(bf),
        "bq8": np.ascontiguousarray(
            (bq[sl] / 8.0).reshape(2, 128).T).astype(np.float32),
        "bkt": np.ascontiguousarray(bk[sl].reshape(2, 128).T).astype(np.float32),
    }


def shared_inputs(q, k, v):
    return {}


# ---------------- public entry point ----------------

_NC_CACHE = []


def _get_nc():
    if not _NC_CACHE:
        _NC_CACHE.append(build())
    return _NC_CACHE[0]


def kernel(q, k, v, Wq, bq, Wk, bk, Wv, bv, Wo, bo):
    from concourse import bass_utils

    args = [np.asarray(a, np.float32) for a in (q, k, v, Wq, bq, Wk, bk, Wv, bv, Wo)]
    q, k, v, Wq, bq, Wk, bk, Wv, bv, Wo = args
    bo = np.asarray(bo, np.float32)

    nc = _get_nc()
    in_maps = [core_inputs(q, k, v, Wq, bq, Wk, bk, Wv, bv, Wo, core)
               for core in range(8)]
    res = bass_utils.run_bass_kernel_spmd(nc, in_maps, core_ids=list(range(8)))

    host_bias = bo.astype(np.float64) + bv.astype(np.float64) @ Wo.astype(np.float64)
    outp = np.zeros((2, T, 1024), np.float64)
    for core in range(8):
        outp[core // 4] += res.results[core]["out"].astype(np.float64)
    return (outp + host_bias).astype(np.float32)
